# revision 28
# baseline (speedup 1.0000x reference)
"""CLIP-style contrastive (NT-Xent) loss on 8 Trainium2 NeuronCores.

Strategy (data-parallel, per sharding hint):
  - Shard the batch (4096) across 8 cores: 512 rows of x_image/x_text each.
  - Each core projects its shard through both towers in TRANSPOSED
    activation layout ([feat_partitions, batch_free]); all operands are
    host-packed into [128, wide] bf16 tensors so every DMA moves >=2KB
    contiguous per partition.
  - The all-gather of the [128, 512] bf16 normalized projections is done
    with direct SBUF->SBUF remote DMA broadcasts (GpSimd SWDGE): every
    core sends its projections to peer (self XOR d) into gather slot d,
    for d=1..7.  All 8 cores sit on one chip (XOR-closed physical tpb
    set), and the similarity row-sums are permutation-invariant, so the
    slot order never needs to be decoded.  This skips the ncfw collective
    machinery (RDH stages + trigger latency) entirely; peers' data lands
    ~2us after it is produced.  A bir-kernel barrier (prelude 1-byte
    AllGather, overlapped with the tower compute) gates the sends so no
    core writes into a peer that has not entered the kernel / cleared its
    semaphores yet.
  - The sim loop runs 5 column groups per 128-row chunk: own columns
    first (no dependency on the gather at all), then image slots 1-7 as
    soon as their arrival semaphore hits, then text slots 1-7.  The 64us
    ScalarE exp tail (the hard throughput floor: ScalarE is the only exp
    engine) therefore starts as early as the data allows and hides the
    gather + the launch skew of the other cores.
  - Device returns raw per-(row,group) partial sums [128, 40] plus
    pos/diag rows [3, 512]; host reduces and finishes in fp64:
        T'_r   = T_r - exp(diag_r/t) + exp(pos_r/t)
        loss_r = log(T'_r) - pos_r/t
    (pos/diag are computed from the bf16-rounded projections exactly like
    the similarity matmul computes those entries, so the big cancellation
    in T' is between nearly identical quantities.)
"""

import numpy as np
import ml_dtypes

import concourse.bacc as bacc
import concourse.bass as bass
import concourse.mybir as mybir
import concourse.tile as tile
from concourse.bass_utils import run_bass_kernel_spmd

NCORES = 8
B, DIN, DE, DH, DP = 4096, 1024, 512, 256, 128
S = B // NCORES            # 512: per-core batch shard
ROWS = 2 * S               # 1024 sim rows owned per core (z1 + z2 shard)
N = 2 * B                  # 8192 global rows
TEMP = 0.07
INV_T = 1.0 / TEMP
KE = DIN // 128            # 8 encoder contraction chunks

F32 = mybir.dt.float32
BF16 = mybir.dt.bfloat16
NPBF = ml_dtypes.bfloat16

# column groups of the sim loop: own (from znb, no gather), img slots 1-3,
# img slots 4-7, txt slots 1-3, txt slots 4-7
GRP_COLS = [1024, 1536, 2048, 1536, 2048]
NG = len(GRP_COLS)

_CACHE: dict = {}


def _build():
    nc = bacc.Bacc("TRN2", target_bir_lowering=False, debug=False,
                   num_devices=NCORES)

    t_in = {}
    for m in ("img", "txt"):
        # per k-chunk: [We_k (512) | xT_k (512)] -> 2KB/partition contiguous
        t_in[f"enc_{m}"] = nc.dram_tensor(f"enc_{m}", [128, KE * 1024], BF16,
                                          kind="ExternalInput")
    # [wp1_img (1024) | wp1_txt (1024) | wp2_img (256) | wp2_txt (256)]
    t_in["wp"] = nc.dram_tensor("wp", [128, 2560], BF16, kind="ExternalInput")
    # [beT_i(4) bp1T_i(2) bp2T_i(1) beT_t(4) bp1T_t(2) bp2T_t(1)]
    t_in["biasT"] = nc.dram_tensor("biasT", [128, 14], F32,
                                   kind="ExternalInput")
    out_stats = nc.dram_tensor("stats", [128, 8 * NG], F32,
                               kind="ExternalOutput")
    out_rows = nc.dram_tensor("rows", [3, S], F32, kind="ExternalOutput")

    # allocate the gather semaphores BEFORE TileContext so its lazy sem
    # allocator can never hand the same numbers to engine-ordering chains
    ag_sems = {m: nc.alloc_semaphore(f"ag_{m}") for m in ("img", "txt")}
    ag_sems["loc"] = nc.alloc_semaphore("ag_loc")

    with tile.TileContext(nc) as tc:
        gates = _emit(nc, tc, t_in, out_stats, out_rows, ag_sems)
    # Attach the cross-core semaphore waits AFTER the Tile scheduling pass:
    # its single-core simulator cannot model remotely-incremented semaphores
    # and would report a deadlock.  generate_event_semaphores (in compile)
    # legalizes instructions that end up with more than one wait.
    trig, gate_mms, sems = gates
    trig._wait_ge(nc._bir_kernel_barrier_sem, nc.bir_kernel_barrier_sem_inc)
    for m in ("img", "txt"):
        gate_mms[m].wait_op(sems[m], 2 * (NCORES - 1), "sem-ge", check=False)
    nc.compile()
    return nc


def _emit(nc, tc, t_in, out_stats, out_rows, ag_sems):
    Exp = mybir.ActivationFunctionType.Exp
    Sqrt = mybir.ActivationFunctionType.Sqrt
    add = mybir.AluOpType.add
    mx = mybir.AluOpType.max

    sem = {m: ag_sems[m] for m in ("img", "txt")}
    sem_loc = ag_sems["loc"]

    with tc.tile_pool(name="const", bufs=1) as cpool, \
         tc.tile_pool(name="encp", bufs=4) as encp, \
         tc.tile_pool(name="wpool", bufs=1) as wpool, \
         tc.tile_pool(name="hg", bufs=2) as hgp, \
         tc.tile_pool(name="actpool", bufs=1) as apool, \
         tc.tile_pool(name="rowsb", bufs=3) as rsb, \
         tc.tile_pool(name="psum", bufs=2, space="PSUM") as pps, \
         tc.tile_pool(name="escp", bufs=2) as escp:

        # fresh semaphores every execution (cleared before the barrier
        # completes, so no peer increment can be wiped)
        for s in (*sem.values(), sem_loc):
            nc.gpsimd.sem_clear(s)
        if nc._bir_kernel_barrier_sem is not None:
            nc.gpsimd.sem_clear(nc._bir_kernel_barrier_sem)

        ones_col = cpool.tile([128, 1], F32)
        nc.any.memset(ones_col[:], 1.0)
        ones_row = cpool.tile([1, 128], F32)
        nc.any.memset(ones_row[:], 1.0)

        # ---- weight/bias/x DMAs: few, wide, two queues ----
        biasT = wpool.tile([128, 14], F32)
        nc.sync.dma_start(out=biasT[:], in_=t_in["biasT"][:, :])
        wp = wpool.tile([128, 2560], BF16)
        nc.scalar.dma_start(out=wp[:], in_=t_in["wp"][:, :])

        enc_tiles = {}
        for m in ("img", "txt"):
            for k in range(KE):
                tl = encp.tile([128, 1024], BF16, tag="enc")
                q = nc.sync if (k % 2 == 0) else nc.scalar
                q.dma_start(out=tl[:], in_=t_in[f"enc_{m}"]
                            [:, 1024 * k:1024 * (k + 1)])
                enc_tiles[(m, k)] = tl

        # gather buffers: one [128, 4096] per tower, peer (self XOR d)'s
        # data lands in slot d (cols 512d..).  Slot 0 is unused for data
        # (own projections are read straight from znb); its first 64 cols
        # are the "arrival gate" scratch: after wait_ge on the arrival
        # semaphore, gpsimd memsets them, a tiny gate matmul reads them,
        # and PE's in-order stream gates every later sim matmul behind it.
        gat = {m: apool.tile([128, 4096], BF16, name=f"g_{m}")
               for m in ("img", "txt")}

        # ---- per-tower: project, normalize, remote-broadcast ----
        znb = {}
        for mi, m in enumerate(("img", "txt")):
            boff = 7 * mi  # bias column offset for this tower
            h_ps = pps.tile([128, 4 * S], F32, tag="simps")
            for k in range(KE):
                tl = enc_tiles[(m, k)]
                for mm in range(4):
                    nc.tensor.matmul(
                        h_ps[:, S * mm:S * (mm + 1)],
                        tl[:, 128 * mm:128 * (mm + 1)],
                        tl[:, 512:1024],
                        start=(k == 0), stop=(k == KE - 1))
            h = hgp.tile([128, 4 * S], BF16, tag="h")
            for mm in range(4):
                nc.vector.tensor_scalar(
                    out=h[:, mm * S:(mm + 1) * S],
                    in0=h_ps[:, mm * S:(mm + 1) * S],
                    scalar1=biasT[:, boff + mm:boff + mm + 1],
                    scalar2=None, op0=add)
            g_ps = pps.tile([128, 2 * S], F32, tag="simps")
            for k2 in range(4):
                for mm2 in range(2):
                    nc.tensor.matmul(
                        g_ps[:, S * mm2:S * (mm2 + 1)],
                        wp[:, 1024 * mi + 256 * k2 + 128 * mm2:
                           1024 * mi + 256 * k2 + 128 * (mm2 + 1)],
                        h[:, S * k2:S * (k2 + 1)],
                        start=(k2 == 0), stop=(k2 == 3))
            g = hgp.tile([128, 2 * S], BF16, tag="g")
            for mm2 in range(2):
                nc.vector.tensor_scalar(
                    out=g[:, mm2 * S:(mm2 + 1) * S],
                    in0=g_ps[:, mm2 * S:(mm2 + 1) * S],
                    scalar1=biasT[:, boff + 4 + mm2:boff + 5 + mm2],
                    scalar2=0.0, op0=add, op1=mx)
            z_ps = pps.tile([128, S], F32, tag="simps")
            for k3 in range(2):
                nc.tensor.matmul(
                    z_ps[:],
                    wp[:, 2048 + 256 * mi + 128 * k3:
                       2048 + 256 * mi + 128 * (k3 + 1)],
                    g[:, S * k3:S * (k3 + 1)],
                    start=(k3 == 0), stop=(k3 == 1))
            z = apool.tile([128, S], F32, name=f"z_{m}")
            nc.vector.tensor_scalar(
                out=z[:], in0=z_ps[:],
                scalar1=biasT[:, boff + 6:boff + 7], scalar2=None, op0=add)

            # rsqrt normalize: DVE reciprocal + ScalarE Sqrt (no table churn)
            sq = rsb.tile([128, S], F32, tag="sq")
            nc.vector.tensor_mul(sq[:], z[:], z[:])
            pssq = pps.tile([1, S], F32, tag="simps")
            nc.tensor.matmul(pssq[:], ones_col[:], sq[:], start=True,
                             stop=True)
            rec = rsb.tile([1, S], F32, tag="rec")
            nc.vector.reciprocal(rec[:], pssq[:])
            inv = rsb.tile([1, S], F32, tag="inv")
            nc.scalar.activation(inv[:], rec[:], Sqrt)
            pinvb = pps.tile([128, S], F32, tag="simps")
            nc.tensor.matmul(pinvb[:], ones_row[:], inv[:], start=True,
                             stop=True)
            zb = apool.tile([128, S], BF16, name=f"znb_{m}")
            nc.vector.tensor_mul(zb[:], z[:], pinvb[:])
            znb[m] = zb

            # remote-broadcast this tower to all 7 peers: peer (self XOR d)
            # receives our projections in slot d of its gather tile.
            for d in range(1, NCORES):
                rdests = [None] * 8
                rdests[d] = (0, d)
                nc.gpsimd.remote_dma_broadcast(
                    out_ap=gat[m][:, 512 * d:512 * (d + 1)], in_ap=zb[:],
                    remote_sem=sem[m], local_sem=sem_loc, rdests=rdests)
            # the trigger "writes" the gate scratch cols (signals_writable)
            # so the gate matmul is dependency-ordered after it
            t = nc.gpsimd.trigger_dma(count=None,
                                      signals_writable=[gat[m][:, 0:64]])
            if mi == 0:
                # the img trigger gets a post-scheduling wait on the
                # bir-kernel barrier: all peers entered + cleared sems
                # before anything flies.  Register the replica group so
                # Bacc inserts the prelude barrier AllGather.
                nc._bir_kernel_barrier_sem_replica_groups.append(
                    set(range(NCORES)))
                trig_img = t

        # ---- pos / self-diag rows from bf16 projections (overlap gather) --
        for r, (a, b) in enumerate((("img", "txt"), ("img", "img"),
                                    ("txt", "txt"))):
            prod = rsb.tile([128, S], F32, tag="prod")
            nc.vector.tensor_mul(prod[:], znb[a][:], znb[b][:])
            pr = pps.tile([1, S], F32, tag="simps")
            nc.tensor.matmul(pr[:], ones_col[:], prod[:], start=True,
                             stop=True)
            row_sb = rsb.tile([1, S], F32, tag="rowsb")
            nc.vector.tensor_copy(row_sb[:], pr[:])
            nc.sync.dma_start(out=out_rows[r:r + 1, :], in_=row_sb[:])

        # ---- main loop: sim rows + exp + fused row sums ----
        # group 0: own columns (znb direct, runs before/while peers land);
        # groups 1-4: gathered slots, gated by a tiny "gate matmul" that
        # reads the trigger-signalled scratch and carries the arrival
        # semaphore wait (PE is in-order, so one gate per tower fences
        # every later matmul).
        gates = {}
        srcs = [(znb["img"], 0, 512, None),
                (gat["img"], 512, 2048, "img"),
                (gat["img"], 2048, 4096, None),
                (gat["txt"], 512, 2048, "txt"),
                (gat["txt"], 2048, 4096, None)]
        stats = apool.tile([128, 8 * NG], F32)
        for tt in range(NG):
            src, c0, c1, gate = srcs[tt]
            w = c1 - c0
            if gate in ("img", "txt"):
                gate_ps = pps.tile([1, 64], F32, tag="simps")
                gates[gate] = nc.tensor.matmul(
                    gate_ps[:], gat[gate][:, 0:1], gat[gate][:, 0:64],
                    start=True, stop=True)
            for rc in range(8):
                if rc < 4:
                    lhs = znb["img"][:, 128 * rc:128 * (rc + 1)]
                else:
                    lhs = znb["txt"][:, 128 * (rc - 4):128 * (rc - 3)]
                ncols = w + (512 if tt == 0 else 0)
                ps = pps.tile([128, ncols], F32, tag="simps")
                for q in range(w // 512):
                    nc.tensor.matmul(
                        ps[:, 512 * q:512 * (q + 1)], lhs,
                        src[:, c0 + 512 * q:c0 + 512 * (q + 1)],
                        start=True, stop=True)
                if tt == 0:
                    nc.tensor.matmul(ps[:, w:w + 512], lhs,
                                     znb["txt"][:, 0:512],
                                     start=True, stop=True)
                esc = escp.tile([128, ncols], BF16, tag="esc")
                nc.scalar.activation(
                    esc[:], ps[:], Exp, scale=INV_T,
                    accum_out=stats[:, NG * rc + tt: NG * rc + tt + 1])

        nc.sync.dma_start(out=out_stats[:, :], in_=stats[:])
        return trig_img, gates, sem


def _prep_in_maps(inputs):
    f32 = np.float32
    host = {}
    # encoder+x chunks (x differs per core; weights shared)
    We = {m: np.asarray(inputs[f"We_{m}"], f32).reshape(KE, 128, DE)
          for m in ("img", "txt")}
    x = {"img": np.asarray(inputs["x_image"], f32),
         "txt": np.asarray(inputs["x_text"], f32)}

    wp_parts = []
    for m in ("img", "txt"):
        wp1 = np.asarray(inputs[f"Wp1_{m}"], f32).reshape(4, 128, DH)
        wp_parts.append(wp1.transpose(1, 0, 2).reshape(128, 4 * DH))
    for m in ("img", "txt"):
        wp2 = np.asarray(inputs[f"Wp2_{m}"], f32).reshape(2, 128, DP)
        wp_parts.append(wp2.transpose(1, 0, 2).reshape(128, 2 * DP))
    host["wp"] = np.ascontiguousarray(
        np.concatenate(wp_parts, axis=1)).astype(NPBF)

    bias_parts = []
    for m in ("img", "txt"):
        bias_parts.append(np.asarray(inputs[f"be_{m}"], f32)
                          .reshape(4, 128).T)
        bias_parts.append(np.asarray(inputs[f"bp1_{m}"], f32)
                          .reshape(2, 128).T)
        bias_parts.append(np.asarray(inputs[f"bp2_{m}"], f32)
                          .reshape(1, 128).T)
    host["biasT"] = np.ascontiguousarray(np.concatenate(bias_parts, axis=1))

    in_maps = []
    for c in range(NCORES):
        mp = dict(host)
        for m in ("img", "txt"):
            xT = np.ascontiguousarray(
                x[m][c * S:(c + 1) * S].T).reshape(KE, 128, S)
            enc = np.concatenate([We[m], xT], axis=2)       # (8,128,1024)
            mp[f"enc_{m}"] = np.ascontiguousarray(
                enc.transpose(1, 0, 2).reshape(128, KE * 1024)).astype(NPBF)
        in_maps.append(mp)
    return in_maps


def _finish_host(results):
    """Host-side fp64 finish: combine per-core stats/rows into the loss."""
    total = 0.0
    t = TEMP
    for c in range(NCORES):
        stats = np.asarray(results[c]["stats"], np.float64)  # [128, 8*NG]
        rows = np.asarray(results[c]["rows"], np.float64)    # [3, 512]
        T = stats.reshape(128, 8, NG).sum(axis=2)            # [128, rc]
        for rc in range(8):
            k = rc % 4
            sl = slice(128 * k, 128 * (k + 1))
            dg = rows[1, sl] if rc < 4 else rows[2, sl]
            pos = rows[0, sl]
            Tp = T[:, rc] - np.exp(dg / t) + np.exp(pos / t)
            total += float(np.sum(np.log(Tp) - pos / t))
    return np.float32(total / N)


def kernel(**inputs) -> np.ndarray:
    nc = _CACHE.get("nc")
    if nc is None:
        nc = _build()
        _CACHE["nc"] = nc
    res = run_bass_kernel_spmd(nc, _prep_in_maps(inputs),
                               core_ids=list(range(NCORES)))
    return _finish_host(res.results)
